# revision 38
# baseline (speedup 1.0000x reference)
"""AdditiveAttention Bass kernel for 8 Trainium2 NeuronCores.

Math (reference):
    q = queries @ W_q            [B,Q,H]
    k = keys @ W_k               [B,K,H]
    scores[b,q,k] = sum_h w_v[h] * tanh(q[b,q,h] + k[b,k,h])
    attn = softmax(mask(scores)) over K
    out = attn @ values          [B,Q,D]

Key structural choices:
  * Masked keys (k >= valid_len[b]) contribute exactly 0 to the softmax, so
    work is skipped at 128-key chunk granularity; valid_lens is host-visible
    inside kernel(), and the work list is built at host (compile) time.
  * |scores| <= ||w_v||_1 ~= 13 so softmax needs no max-subtraction; the
    per-chunk partials (o = sum exp(s)*v, z = sum exp(s)) are linear and are
    summed on host.
  * Valid keys are chunked at 128 granularity; chunks are packed into uniform
    per-core slots: same-batch chunk PAIRS become C=256 tasks (fewer, bigger
    DVE instructions) and leftovers become C=128 tasks.  Every core runs the
    identical program (SPMD); load balance is exact by construction.

Per-task device pipeline (C = task keys, in CH chunks of 128):
    PE : q_proj/k_proj projections (H on partitions)   [pipelined 1 task ahead]
    DVE: qk[h, q, c] = k_proj[h, c] + q_proj[h, q]     (per-partition scalar)
    ACT: feat = tanh(qk) -> bf16, flat 8K-element instructions
    PE : scoresT[c, q] = feat.T @ w_v                  (feat as stationary)
    ACT: p = exp(scoresT)
    PE : o[d, q] = V.T @ p ; z[q] = mask.T @ p         (mask via zeroed V rows)
Host: out[b] = (sum_t o_t) / (sum_t z_t).
"""

import math
from contextlib import ExitStack

import numpy as np
import ml_dtypes

import concourse.bass as bass
import concourse.mybir as mybir
import concourse.tile as tile
from concourse import bacc, bass_utils

F32 = mybir.dt.float32
F16 = mybir.dt.float16

B, Q, K, D, H = 16, 64, 1024, 256, 256
CG = 128         # chunk granularity
GQ = 16          # queries per tanh group
N_CORES = 8
DC = D // 128    # d chunks (2)
HC = H // 128    # h chunks (2)


def emit_kernel(tc, aps, slot_cs):
    """Emit the per-core SPMD program; slot_cs[t] = C of slot t."""
    nc = tc.nc
    ctx = tc.ctx
    n_tasks = len(slot_cs)

    Wq = aps["Wq"]              # [128, DC, H] f32      (dp, dc, h)
    Wk = aps["Wk"]
    wv = aps["wv"]              # [128, HC] bf16

    const_pool = ctx.enter_context(tc.tile_pool(name="const", bufs=1))
    in_pool = ctx.enter_context(tc.tile_pool(name="inp", bufs=2))
    proj_pool = ctx.enter_context(tc.tile_pool(name="proj", bufs=2))
    qk_pool = ctx.enter_context(tc.tile_pool(name="qk", bufs=3))
    feat_pool = ctx.enter_context(tc.tile_pool(name="feat", bufs=4))
    p_pool = ctx.enter_context(tc.tile_pool(name="p", bufs=2))
    out_pool = ctx.enter_context(tc.tile_pool(name="outp", bufs=2))
    ps_proj = ctx.enter_context(tc.tile_pool(name="psproj", bufs=2, space="PSUM"))
    ps_sc = ctx.enter_context(tc.tile_pool(name="pssc", bufs=2, space="PSUM"))
    ps_o = ctx.enter_context(tc.tile_pool(name="pso", bufs=2, space="PSUM"))

    Wq_sb = const_pool.tile([128, DC, H], F16, tag="wq")
    Wk_sb = const_pool.tile([128, DC, H], F16, tag="wk")
    wv_sb = const_pool.tile([128, HC], F16, tag="wv")
    nc.sync.dma_start(Wq_sb[:], Wq[:])
    nc.sync.dma_start(Wk_sb[:], Wk[:])
    nc.gpsimd.dma_start(wv_sb[:], wv[:])

    # PE warm-up: dummy matmuls with no DMA dependency, so the HAM clock gate
    # opens during the initial DMA window instead of during the first
    # projections.
    warm = const_pool.tile([128, 128], F16, tag="warm")
    warm_ps = ps_o.tile([128, DC, Q], F32, tag="o")
    nc.vector.memset(warm[:], 0.0)
    for r in range(30):
        nc.tensor.matmul(warm_ps[:, 0, :], lhsT=warm[:], rhs=warm[:, 0:Q],
                         start=True, stop=True)

    def emit_inputs_and_proj(t):
        """DMA inputs + projections + evacuation for slot t."""
        C = slot_cs[t]
        CH = C // 128
        projw = Q + C if HC * (Q + C) * 4 <= 2048 else 512
        k_sb = in_pool.tile([128, DC, C], F16, tag="k")
        qT_sb = in_pool.tile([128, DC, Q], F16, tag="q")
        v_sb = in_pool.tile([128, CH, D], F32, tag="v")
        m_sb = in_pool.tile([128, CH], F32, tag="m")
        nc.sync.dma_start(qT_sb[:], aps[f"queriesT{t}"])
        for dc in range(DC):
            nc.sync.dma_start(k_sb[:, dc], aps[f"keysT{t}"][:, dc])
        nc.gpsimd.dma_start(m_sb[:], aps[f"maskv{t}"])
        nc.gpsimd.dma_start(v_sb[:], aps[f"vals{t}"])

        # proj_ps[:, hh, 0:Q] = q_proj; [:, hh, Q:Q+C] = k_proj
        # (per-hh region inside one PSUM bank when it fits)
        proj_ps = ps_proj.tile([128, HC, projw], F32, tag="proj")
        for hh in range(HC):
            for dc in range(DC):
                nc.tensor.matmul(
                    proj_ps[:, hh, 0:Q],
                    lhsT=Wq_sb[:, dc, hh * 128:(hh + 1) * 128],
                    rhs=qT_sb[:, dc, :],
                    start=(dc == 0), stop=(dc == DC - 1),
                )
            for dc in range(DC):
                nc.tensor.matmul(
                    proj_ps[:, hh, Q:Q + C],
                    lhsT=Wk_sb[:, dc, hh * 128:(hh + 1) * 128],
                    rhs=k_sb[:, dc, :],
                    start=(dc == 0), stop=(dc == DC - 1),
                )
        qp_sb = proj_pool.tile([128, HC, Q], F32, tag="qp")
        kp_sb = proj_pool.tile([128, HC * C], F16, tag="kp")
        nc.vector.tensor_copy(qp_sb[:], proj_ps[:, :, 0:Q])
        nc.vector.tensor_copy(
            kp_sb[:].rearrange("p (h c) -> p h c", h=HC),
            proj_ps[:, :, Q:Q + C])
        return k_sb, qT_sb, v_sb, m_sb, qp_sb, kp_sb

    state = {}

    for t in range(n_tasks):
        C = slot_cs[t]
        CH = C // 128
        if t == 0:
            state[0] = emit_inputs_and_proj(0)
        _, _, v_sb, m_sb, qp_sb, kp_sb = state.pop(t)
        if t + 1 < n_tasks:
            # Pipelined: next task's projections go ahead of this task's
            # scores in the PE stream, so PE/DVE never stall at task turnover.
            state[t + 1] = emit_inputs_and_proj(t + 1)

        # ---- qk broadcast-add (DVE) + tanh (ACT), flat tiles ----
        if t == 0:
            group_lens = [2, 6, 8] + [GQ] * ((Q - GQ) // GQ)
        elif t == n_tasks - 1:
            group_lens = [GQ] * ((Q - GQ) // GQ) + [GQ - 8, 4, 4]
        else:
            group_lens = [GQ] * (Q // GQ)
        feats = []   # (flat feat tile, local idx) per query
        q0 = 0
        for ln in group_lens:
            qk = qk_pool.tile([128, GQ * HC * C], F16, tag="qk")
            for i in range(ln):
                qq = q0 + i
                for hh in range(HC):
                    nc.vector.tensor_scalar_add(
                        qk[:, (i * HC + hh) * C:(i * HC + hh + 1) * C],
                        kp_sb[:, hh * C:(hh + 1) * C],
                        qp_sb[:, hh, qq:qq + 1],
                    )
            feat = feat_pool.tile([128, GQ * HC * C], F16, tag="feat")
            nc.scalar.activation(feat[:, 0:ln * HC * C], qk[:, 0:ln * HC * C],
                                 mybir.ActivationFunctionType.Tanh)
            for i in range(ln):
                feats.append((feat, i))
            q0 += ln

        # ---- scoresT[c, q] (PE): feat as stationary, w_v streaming ----
        sc_ps = ps_sc.tile([128, (CH + 1) * Q], F32, tag="sc")
        for qq in range(Q):
            ftile, i = feats[qq]
            for ch in range(CH):
                for hh in range(HC):
                    off = (i * HC + hh) * C + ch * 128
                    nc.tensor.matmul(
                        sc_ps[:, ch * Q + qq:ch * Q + qq + 1],
                        lhsT=ftile[:, off:off + 128],
                        rhs=wv_sb[:, hh:hh + 1],
                        start=(hh == 0), stop=(hh == HC - 1),
                    )

        # ---- exp (ACT) ----
        p_sb = p_pool.tile([128, CH * Q], F32, tag="p")
        nc.scalar.activation(p_sb[:], sc_ps[:, 0:CH * Q],
                             mybir.ActivationFunctionType.Exp)

        # ---- o = V.T @ p, z = mask.T @ p (PE, accumulate over ch) ----
        o_ps = ps_o.tile([128, DC, Q], F32, tag="o")
        for dc in range(DC):
            for ch in range(CH):
                nc.tensor.matmul(
                    o_ps[:, dc, :],
                    lhsT=v_sb[:, ch, dc * 128:(dc + 1) * 128],
                    rhs=p_sb[:, ch * Q:(ch + 1) * Q],
                    start=(ch == 0), stop=(ch == CH - 1),
                )
        for ch in range(CH):
            nc.tensor.matmul(
                sc_ps[0:1, CH * Q:(CH + 1) * Q],
                lhsT=m_sb[:, ch:ch + 1],
                rhs=p_sb[:, ch * Q:(ch + 1) * Q],
                start=(ch == 0), stop=(ch == CH - 1),
            )

        # ---- evacuate + output DMA ----
        o_sb = out_pool.tile([128, DC, Q], F32, tag="osb")
        s_sb = out_pool.tile([1, Q], F32, tag="ssb")
        nc.vector.tensor_copy(o_sb[:], o_ps[:])
        nc.vector.tensor_copy(s_sb[:], sc_ps[0:1, CH * Q:(CH + 1) * Q])
        nc.sync.dma_start(aps[f"o_out{t}"], o_sb[:])
        nc.sync.dma_start(aps[f"s_out{t}"], s_sb[:])


_NC_CACHE = {}


def build_nc(slot_cs):
    key = tuple(slot_cs)
    if key in _NC_CACHE:
        return _NC_CACHE[key]
    nc = bacc.Bacc("TRN2", target_bir_lowering=False, debug=False)
    aps = {
        "Wq": nc.dram_tensor("Wq", [128, DC, H], F16, kind="ExternalInput").ap(),
        "Wk": nc.dram_tensor("Wk", [128, DC, H], F16, kind="ExternalInput").ap(),
        "wv": nc.dram_tensor("wv", [128, HC], F16, kind="ExternalInput").ap(),
    }
    for t, C in enumerate(slot_cs):
        CH = C // 128
        aps[f"keysT{t}"] = nc.dram_tensor(
            f"keysT{t}", [128, DC, C], F16, kind="ExternalInput").ap()
        aps[f"queriesT{t}"] = nc.dram_tensor(
            f"queriesT{t}", [128, DC, Q], F16, kind="ExternalInput").ap()
        aps[f"vals{t}"] = nc.dram_tensor(
            f"vals{t}", [128, CH, D], F32, kind="ExternalInput").ap()
        aps[f"maskv{t}"] = nc.dram_tensor(
            f"maskv{t}", [128, CH], F32, kind="ExternalInput").ap()
        aps[f"o_out{t}"] = nc.dram_tensor(
            f"o_out{t}", [128, DC, Q], F32, kind="ExternalOutput").ap()
        aps[f"s_out{t}"] = nc.dram_tensor(
            f"s_out{t}", [1, Q], F32, kind="ExternalOutput").ap()
    with tile.TileContext(nc) as tc:
        with ExitStack() as stack:
            tc.ctx = stack
            emit_kernel(tc, aps, slot_cs)
    nc.compile()
    _NC_CACHE[key] = (nc, aps)
    return nc, aps


def make_task_list(valid_lens):
    """Pack 128-key chunks into per-core slots.

    Returns (per_core, slot_cs): per_core[core][t] = (b, [c0, ...]) with
    len(c0s) == slot_cs[t] // CG chunks, all from batch b, or None (dummy).
    """
    pairs = []    # (b, [c0a, c0b])
    singles = []  # (b, [c0])
    for b in range(B):
        v = int(valid_lens[b])
        c0s = list(range(0, v, CG))
        while len(c0s) >= 2:
            pairs.append((b, [c0s.pop(0), c0s.pop(0)]))
        if c0s:
            singles.append((b, [c0s.pop(0)]))

    total = 2 * len(pairs) + len(singles)
    total_pad = math.ceil(total / N_CORES) * N_CORES
    chunks_pc = total_pad // N_CORES
    nd, ns = divmod(chunks_pc, 2)
    # Need N_CORES*nd pairs and N_CORES*ns singles; convert pairs <-> singles
    # (pair -> 2 singles always possible; singles -> pair only if same b).
    need_p, need_s = N_CORES * nd, N_CORES * ns
    while len(pairs) > need_p:
        b, (c0a, c0b) = pairs.pop()
        singles += [(b, [c0a]), (b, [c0b])]
    while len(singles) < need_s:
        singles.append(None)   # dummy single
    if len(pairs) < need_p:
        # Not enough same-b pairs: top up with dummy pairs if the singles
        # count already matches, else fall back to uniform-C=256 chunking.
        deficit = need_p - len(pairs)
        if len(singles) == need_s:
            pairs += [None] * deficit
        else:
            # fallback: uniform 256 chunking
            chunks = []
            for b in range(B):
                v = int(valid_lens[b])
                for c0 in range(0, v, 2 * CG):
                    chunks.append((b, [c0, c0 + CG]))
            n_tasks = math.ceil(len(chunks) / N_CORES)
            chunks += [None] * (n_tasks * N_CORES - len(chunks))
            per_core = [chunks[i * n_tasks:(i + 1) * n_tasks]
                        for i in range(N_CORES)]
            return per_core, [2 * CG] * n_tasks
    # duals first (big groups saturate ACT fastest); single last (short tail)
    slot_cs = [2 * CG] * nd + [CG] * ns
    per_core = []
    for i in range(N_CORES):
        row = pairs[i * nd:(i + 1) * nd] + singles[i * ns:(i + 1) * ns]
        per_core.append(row)
    return per_core, slot_cs


def pack_inputs(queries, keys, values, valid_lens, W_q, W_k, w_v,
                per_core, slot_cs):
    """Build the per-core input maps (host-side layout only)."""
    BFD = np.float16
    Wq_arr = np.ascontiguousarray(
        W_q.reshape(DC, 128, H).transpose(1, 0, 2)).astype(BFD)  # [128, DC, H]
    Wk_arr = np.ascontiguousarray(
        W_k.reshape(DC, 128, H).transpose(1, 0, 2)).astype(BFD)
    wv_arr = np.ascontiguousarray(
        w_v.reshape(HC, 128).T.astype(BFD))                      # [128, HC]

    in_maps = []
    for core in range(N_CORES):
        m = {"Wq": Wq_arr, "Wk": Wk_arr, "wv": wv_arr}
        for t, C in enumerate(slot_cs):
            CH = C // 128
            keysT = np.zeros((128, DC, C), BFD)
            queriesT = np.zeros((128, DC, Q), BFD)
            vals = np.zeros((128, CH, D), np.float32)
            maskv = np.zeros((128, CH), np.float32)
            task = per_core[core][t]
            if task is not None:
                b, c0s = task
                v = int(valid_lens[b])
                kT = np.zeros((D, C), np.float32)
                vv = np.zeros((C, D), np.float32)
                mm = np.zeros(C, np.float32)
                for j, c0 in enumerate(c0s):
                    n = min(CG, v - c0)
                    kT[:, j * CG:j * CG + n] = keys[b, c0:c0 + n, :].T
                    vv[j * CG:j * CG + n] = values[b, c0:c0 + n, :]
                    mm[j * CG:j * CG + n] = 1.0
                keysT[:] = kT.reshape(DC, 128, C).transpose(1, 0, 2)
                queriesT[:] = queries[b].T.reshape(DC, 128, Q).transpose(1, 0, 2)
                vals[:] = vv.reshape(CH, 128, D).transpose(1, 0, 2)
                maskv[:] = mm.reshape(CH, 128).T
            m[f"keysT{t}"] = keysT
            m[f"queriesT{t}"] = queriesT
            m[f"vals{t}"] = vals
            m[f"maskv{t}"] = maskv
        in_maps.append(m)
    return in_maps


def combine_outputs(results, per_core, slot_cs):
    o_acc = np.zeros((B, D, Q), np.float64)
    s_acc = np.zeros((B, Q), np.float64)
    for core in range(N_CORES):
        for t in range(len(slot_cs)):
            task = per_core[core][t]
            if task is None:
                continue
            b, _ = task
            o = results[core][f"o_out{t}"]   # [128, DC, Q]
            s = results[core][f"s_out{t}"]   # [1, Q]
            o_acc[b] += o.transpose(1, 0, 2).reshape(D, Q)
            s_acc[b] += s[0]
    out = o_acc / s_acc[:, None, :]          # [B, D, Q]
    return np.ascontiguousarray(out.transpose(0, 2, 1)).astype(np.float32)


def kernel(queries, keys, values, valid_lens, W_q, W_k, w_v, _run_kwargs=None):
    queries = np.asarray(queries, np.float32)
    keys = np.asarray(keys, np.float32)
    values = np.asarray(values, np.float32)
    valid_lens = np.asarray(valid_lens)
    W_q = np.asarray(W_q, np.float32)
    W_k = np.asarray(W_k, np.float32)
    w_v = np.asarray(w_v, np.float32)

    per_core, slot_cs = make_task_list(valid_lens)
    nc, _ = build_nc(slot_cs)
    in_maps = pack_inputs(queries, keys, values, valid_lens, W_q, W_k, w_v,
                          per_core, slot_cs)
    kw = dict(_run_kwargs or {})
    res = bass_utils.run_bass_kernel_spmd(nc, in_maps, list(range(N_CORES)), **kw)
    out = combine_outputs(res.results, per_core, slot_cs)
    if _run_kwargs is not None:
        kernel._last_result = res
    return out
